# revision 33
# baseline (speedup 1.0000x reference)
"""Distributed Trainium2 Bass kernel for a full attention layer (prefill).

Reference computation (B=4, S=1024, D=4096, H=32, HD=128, fp32 I/O):
    xq = rope(x @ wq.T), xk = rope(x @ wk.T), xv = x @ wv.T
    out = softmax(causal(xq xk^T / sqrt(HD))) @ xv
    y   = out @ wo.T

Sharding: 8-way tensor parallel over heads (4 heads / core).  Each core
computes Q/K/V projections for its heads, attention, then the output
projection for its 512-row slice of y^T after an AllGather of the
per-head attention outputs.  Matmul operands are cast to bf16 on the
host (fp32 PSUM accumulation on chip).

Layout notes (everything "transposed", i.e. feature-major):
  - x is fed as xT [D, B*S] so projections produce qT/kT [o, tok]
    directly and the attention chain needs no on-chip transposes:
      scoresT[j,i] = kT_tile^T @ qT_chunk        (j keys, i queries)
      softmax over j (partition axis) via exp + ones-matmul column sums
      attn_T[hd,i] = v_tile^T(nat) @ probsT      (v kept token-major)
  - RoPE pairs are split (re | im halves) per head by permuting
    wq/wk rows on the host.  The cross-partition half-swap is done with
    two SBUF->SBUF DMAs (walrus forbids DVE ops whose operands start at
    different partitions), then the rotation is 3 full-width DVE ops:
      new = q * [c;c] + swap(q) * [-s;s]
  - Causal mask: block-skip fully-masked (j,i) tiles; additive band
    mask (4 distinct 128x512 patterns) on diagonal-crossing tiles.
  - Softmax skips max-subtraction: scores ~ N(0,1), |score| < ~7,
    exp is safe in fp32.  1/sum via reciprocal_approx_fast (~18 bits)
    + gpsimd partition_broadcast.

Schedule: P(ch0..7) -> A(b0) A(b1) W(b0) A(b2) W(b1) A(b3) W(b2) W(b3)
with AllGather(b) issued at the end of A(b); W(b) consumes agout(b).
PV work is software-pipelined one head behind scores/exp so the PE
never waits on the Scalar engine's exp.
"""

import math
import os
import sys

import numpy as np

for _p in ("/opt/trn_rl_repo", "/root/.axon_site/_ro/trn_rl_repo"):
    if os.path.isdir(_p) and _p not in sys.path:
        sys.path.insert(0, _p)

import ml_dtypes  # noqa: E402
import concourse.bass as bass  # noqa: E402
import concourse.mybir as mybir  # noqa: E402
import concourse.tile as tile  # noqa: E402
from concourse import bacc  # noqa: E402
from concourse.bass_utils import run_bass_kernel_spmd  # noqa: E402

B, S, D, H = 4, 1024, 4096, 32
HD = D // H            # 128
NC = 8                 # cores
HPC = H // NC          # 4 heads per core
OC = HPC * HD          # 512 output dims per core
NT = B * S             # 4096 tokens
P = 128
KT = D // P            # 32 contraction tiles
KP = 2                 # k-parts per chunk DMA (for startup pipelining)
KTP = KT // KP         # 16 k-tiles per part
TCH = 512              # token chunk (columns per projection matmul)
NCH = NT // TCH        # 8 chunks
SCALE = 1.0 / math.sqrt(HD)

BF16 = mybir.dt.bfloat16
F32 = mybir.dt.float32


def build():
    nc = bacc.Bacc("TRN2", target_bir_lowering=False, debug=False,
                   num_devices=NC)

    # ---- I/O ----
    # x and weights arrive pre-tiled to the exact SBUF image so their
    # DMAs are fully contiguous (16KB/partition descriptors)
    xT_d = nc.dram_tensor("xT", [NCH, P, KT, TCH], BF16,
                          kind="ExternalInput")
    wqT_d = nc.dram_tensor("wqT", [P, KT, OC], BF16, kind="ExternalInput")
    wkT_d = nc.dram_tensor("wkT", [P, KT, OC], BF16, kind="ExternalInput")
    wvT_d = nc.dram_tensor("wvT", [P, KT, OC], BF16, kind="ExternalInput")
    woT_d = nc.dram_tensor("woT", [P, KT, OC], BF16, kind="ExternalInput")
    ccT_d = nc.dram_tensor("ccT", [P, S], F32, kind="ExternalInput")
    ssT_d = nc.dram_tensor("ssT", [P, S], F32, kind="ExternalInput")
    mb_d = nc.dram_tensor("mband", [P, 4 * TCH], F32, kind="ExternalInput")
    out_d = nc.dram_tensor("out", [OC, NT], F32, kind="ExternalOutput")

    # ---- internal DRAM ----
    qT_d = nc.dram_tensor("qT_spill", [OC, NT], BF16)
    kT_d = nc.dram_tensor("kT_spill", [OC, NT], BF16)
    v_d = nc.dram_tensor("v_spill", [NT, OC], BF16)
    agin = [nc.dram_tensor(f"agin{b}", [OC, S], BF16) for b in range(B)]
    agout = [nc.dram_tensor(f"agout{b}", [D, S], BF16, addr_space="Shared")
             for b in range(B)]

    def part(dram_ap, csl, kp):
        """k-part kp of a [D, n] dram tensor column slice as [P, KTP, n]."""
        ksl = slice(kp * KTP * P, (kp + 1) * KTP * P)
        return dram_ap[ksl, csl].rearrange("(k p) n -> p k n", p=P)

    def wpart(dram_ap, kp):
        """k-part kp of a pre-tiled [P, KT, n] weight tensor."""
        return dram_ap[:, kp * KTP:(kp + 1) * KTP, :]

    with tile.TileContext(nc) as tc, \
         tc.tile_pool(name="const", bufs=1) as cpool:
        # constants on the gpsimd DMA queue (off the critical path);
        # const pool stays live across all phases
        ccT = cpool.tile([P, S], F32, tag="cc")
        ssT = cpool.tile([P, S], F32, tag="ss")
        mband = cpool.tile([P, 4 * TCH], F32, tag="mb")
        ones_col = cpool.tile([P, 1], BF16, tag="oc")
        nc.gpsimd.dma_start(ccT[:], ccT_d.ap())
        nc.gpsimd.dma_start(ssT[:], ssT_d.ap())
        nc.gpsimd.dma_start(mband[:], mb_d.ap())
        nc.vector.memset(ones_col[:], 1.0)

        with tc.tile_pool(name="pw", bufs=1) as pw, \
             tc.tile_pool(name="px", bufs=4) as px, \
             tc.tile_pool(name="pr", bufs=3) as pr, \
             tc.tile_pool(name="po", bufs=4) as po, \
             tc.tile_pool(name="pps", bufs=4, space="PSUM") as pps:

            # weights as 2 k-parts each; only wq part0 ahead of the first
            # x chunk (x streams on the Scalar engine's separate HWDGE ring,
            # so weight and x loads run concurrently)
            w_sb = {}
            for kp in range(KP):
                for wname, wd in (("q", wqT_d), ("k", wkT_d), ("v", wvT_d)):
                    t = pw.tile([P, KTP, OC], BF16, tag=f"w{wname}{kp}")
                    w_sb[(wname, kp)] = (t, wd)
            order = [("q", 0), ("q", 1), ("k", 0), ("k", 1), ("v", 0),
                     ("v", 1)]
            for wname, kp in order:
                t, wd = w_sb[(wname, kp)]
                nc.sync.dma_start(t[:], wpart(wd.ap(), kp))
                w_sb[(wname, kp)] = t

            # ---------- phase P: projections + RoPE ----------
            def proj_chunk(ch):
                tsl = slice(ch * TCH, (ch + 1) * TCH)
                psl = slice((ch % 2) * TCH, (ch % 2) * TCH + TCH)
                xc = [px.tile([P, KTP, TCH], BF16, tag="x", name=f"xc{kp}")
                      for kp in range(KP)]
                for kp in range(KP):
                    nc.scalar.dma_start(
                        xc[kp][:],
                        xT_d.ap()[ch, :, kp * KTP:(kp + 1) * KTP, :])

                # q/k projections (transposed out) + RoPE
                for wname, spill in (("q", qT_d), ("k", kT_d)):
                    for ot in range(HPC):
                        osl = slice(ot * P, (ot + 1) * P)
                        ps = pps.tile([P, TCH], F32, tag="ps")
                        for k in range(KT):
                            kp, ki = divmod(k, KTP)
                            nc.tensor.matmul(
                                ps[:], lhsT=w_sb[(wname, kp)][:, ki, osl],
                                rhs=xc[kp][:, ki, :],
                                start=(k == 0), stop=(k == KT - 1))
                        qb = pr.tile([P, TCH], BF16, tag="qb")
                        nc.scalar.copy(qb[:], ps[:])
                        sw = pr.tile([P, TCH], BF16, tag="sw")
                        nc.sync.dma_start(sw[0:64, :], qb[64:128, :])
                        nc.sync.dma_start(sw[64:128, :], qb[0:64, :])
                        qc = pr.tile([P, TCH], F32, tag="qc")
                        qs = pr.tile([P, TCH], F32, tag="qs")
                        nc.vector.tensor_tensor(
                            out=qc[:], in0=ps[:], in1=ccT[:, psl],
                            op=mybir.AluOpType.mult)
                        nc.vector.tensor_tensor(
                            out=qs[:], in0=sw[:], in1=ssT[:, psl],
                            op=mybir.AluOpType.mult)
                        ro = pr.tile([P, TCH], BF16, tag="ro")
                        nc.vector.tensor_tensor(
                            out=ro[:], in0=qc[:], in1=qs[:],
                            op=mybir.AluOpType.add)
                        nc.sync.dma_start(spill.ap()[osl, tsl], ro[:])

                # v projection (token-major out)
                for jt in range(TCH // P):
                    jsl = slice(jt * P, (jt + 1) * P)
                    ps = pps.tile([P, OC], F32, tag="ps")
                    for k in range(KT):
                        kp, ki = divmod(k, KTP)
                        nc.tensor.matmul(
                            ps[:], lhsT=xc[kp][:, ki, jsl],
                            rhs=w_sb[("v", kp)][:, ki, :],
                            start=(k == 0), stop=(k == KT - 1))
                    vo = po.tile([P, OC], BF16, tag="vo")
                    nc.vector.tensor_copy(vo[:], ps[:])
                    nc.sync.dma_start(
                        v_d.ap()[ch * TCH + jt * P:
                                 ch * TCH + (jt + 1) * P, :], vo[:])

            for ch in range(NCH):
                proj_chunk(ch)

        # ---------- phases A (attention) + W (output proj), interleaved ----
        with tc.tile_pool(name="aq", bufs=4) as aq, \
             tc.tile_pool(name="av", bufs=6) as av, \
             tc.tile_pool(name="app", bufs=2) as app, \
             tc.tile_pool(name="ao", bufs=3) as ao, \
             tc.tile_pool(name="asm", bufs=4) as asm, \
             tc.tile_pool(name="ww", bufs=1) as ww, \
             tc.tile_pool(name="wg", bufs=4) as wg, \
             tc.tile_pool(name="wy", bufs=4) as wy, \
             tc.tile_pool(name="aps", bufs=2, space="PSUM") as aps, \
             tc.tile_pool(name="apv", bufs=2, space="PSUM") as apv, \
             tc.tile_pool(name="asu", bufs=2, space="PSUM") as asu, \
             tc.tile_pool(name="wps", bufs=2, space="PSUM") as wps:

            # wo loads on the scalar ring: it is idle at attention start,
            # and the sync ring must stay clear for q/k/v loads
            wo_sb = {}
            for kp in range(KP):
                t = ww.tile([P, KTP, OC], BF16, tag=f"wo{kp}")
                nc.scalar.dma_start(t[:], wpart(woT_d.ap(), kp))
                wo_sb[kp] = t

            def jmax(ic):       # causal: j tiles 0..jmax-1 for i-chunk ic
                return 4 * (ic + 1)

            def mslot(ic, jt):  # probsT slot index
                return jt if ic == 0 else 4 + jt

            def do_scores(b, h):
                qTh = aq.tile([P, S], BF16, tag="q")
                kTh = aq.tile([P, S], BF16, tag="k")
                vh = av.tile([P, S // P, HD], BF16, tag="v")
                hsl = slice(h * P, (h + 1) * P)
                bsl = slice(b * S, (b + 1) * S)
                nc.sync.dma_start(qTh[:], qT_d.ap()[hsl, bsl])
                nc.sync.dma_start(kTh[:], kT_d.ap()[hsl, bsl])
                nc.sync.dma_start(
                    vh[:],
                    v_d.ap()[bsl, hsl].rearrange("(j p) n -> p j n", p=P))
                pp = app.tile([P, 12, TCH], BF16, tag="pp")
                for ic in range(2):
                    for jt in range(jmax(ic)):
                        m = mslot(ic, jt)
                        r = jt - 4 * ic
                        if r < 0:
                            # fully below the diagonal: no mask
                            sps = aps.tile([P, TCH], F32, tag="s")
                            nc.tensor.matmul(
                                sps[:], lhsT=kTh[:, jt * P:(jt + 1) * P],
                                rhs=qTh[:, ic * TCH:(ic + 1) * TCH],
                                start=True, stop=True)
                            nc.scalar.activation(
                                pp[:, m, :], sps[:],
                                mybir.ActivationFunctionType.Exp,
                                scale=SCALE)
                        else:
                            # diagonal band: cols [0,128r) fully masked ->
                            # memset; triangular mask only on the 128-col
                            # diagonal slice; skip matmul on masked cols
                            w = TCH - r * P   # live columns
                            sps = aps.tile([P, TCH], F32, tag="s")
                            nc.tensor.matmul(
                                sps[:, :w], lhsT=kTh[:, jt * P:(jt + 1) * P],
                                rhs=qTh[:, ic * TCH + r * P:(ic + 1) * TCH],
                                start=True, stop=True)
                            nc.vector.tensor_tensor(
                                out=sps[:, 0:P], in0=sps[:, 0:P],
                                in1=mband[:, 0:P],
                                op=mybir.AluOpType.add)
                            if r > 0:
                                nc.gpsimd.memset(pp[:, m, 0:r * P], 0.0)
                            nc.scalar.activation(
                                pp[:, m, r * P:], sps[:, :w],
                                mybir.ActivationFunctionType.Exp,
                                scale=SCALE)
                return pp, vh

            def do_pv(b, h, pp, vh):
                at = ao.tile([P, S], BF16, tag="at")
                for ic in range(2):
                    pv = apv.tile([P, TCH], F32, tag="pv")
                    su = asu.tile([1, TCH], F32, tag="su")
                    jm = jmax(ic)
                    for jt in range(jm):
                        nc.tensor.matmul(
                            pv[:], lhsT=vh[:, jt, :],
                            rhs=pp[:, mslot(ic, jt), :],
                            start=(jt == 0), stop=(jt == jm - 1))
                    for jt in range(jm):
                        nc.tensor.matmul(
                            su[:], lhsT=ones_col[:],
                            rhs=pp[:, mslot(ic, jt), :],
                            start=(jt == 0), stop=(jt == jm - 1))
                    rec = asm.tile([1, TCH], F32, tag="rec")
                    nc.vector.reciprocal_approx_fast(rec[:], su[:])
                    bcast = asm.tile([P, TCH], F32, tag="bcast")
                    nc.gpsimd.partition_broadcast(bcast[:], rec[:])
                    nc.vector.tensor_tensor(
                        out=at[:, ic * TCH:(ic + 1) * TCH], in0=pv[:],
                        in1=bcast[:], op=mybir.AluOpType.mult)
                nc.sync.dma_start(agin[b].ap()[h * P:(h + 1) * P, :], at[:])
                if h == HPC - 1:
                    nc.gpsimd.collective_compute(
                        "AllGather", mybir.AluOpType.bypass,
                        ins=[agin[b].ap().opt()],
                        outs=[agout[b].ap().opt()],
                        replica_groups=[list(range(NC))])

            def do_attn(b):
                prev = None
                for h in range(HPC):
                    pp, vh = do_scores(b, h)
                    if prev is not None:
                        do_pv(*prev)
                    prev = (b, h, pp, vh)
                do_pv(*prev)

            def do_wo(b):
                for tc2 in range(2):
                    ch = b * 2 + tc2
                    csl = slice(tc2 * TCH, (tc2 + 1) * TCH)
                    agc = [wg.tile([P, KTP, TCH], BF16, tag="ag",
                                   name=f"agc{kp}") for kp in range(KP)]
                    for kp in range(KP):
                        nc.scalar.dma_start(agc[kp][:],
                                            part(agout[b].ap(), csl, kp))
                    for ot in range(HPC):
                        osl = slice(ot * P, (ot + 1) * P)
                        ps = wps.tile([P, TCH], F32, tag="ps")
                        for k in range(KT):
                            kp, ki = divmod(k, KTP)
                            nc.tensor.matmul(
                                ps[:], lhsT=wo_sb[kp][:, ki, osl],
                                rhs=agc[kp][:, ki, :],
                                start=(k == 0), stop=(k == KT - 1))
                        yt = wy.tile([P, TCH], F32, tag="y")
                        nc.vector.tensor_copy(yt[:], ps[:])
                        nc.sync.dma_start(
                            out_d.ap()[osl, ch * TCH:(ch + 1) * TCH], yt[:])

            for b in range(B):
                do_attn(b)
            for b in range(B):
                do_wo(b)

    nc.compile()
    return nc


_BUILT = {}


def _get_nc():
    if "nc" not in _BUILT:
        _BUILT["nc"] = build()
    return _BUILT["nc"]


def _tile_w(w_slice):
    """[OC, D] weight slice -> pre-tiled lhsT image [P, KT, OC] bf16."""
    return np.ascontiguousarray(
        w_slice.T.reshape(KT, P, OC).transpose(1, 0, 2)
        .astype(ml_dtypes.bfloat16))


def _prep_inputs(x, wq, wk, wv, wo, freqs_cos, freqs_sin, mask):
    bf = ml_dtypes.bfloat16
    # x -> [NCH, P, KT, TCH] with xtc[ch, p, k, n] = x[512ch+n, 128k+p]
    xT = np.ascontiguousarray(
        np.asarray(x).reshape(NCH, TCH, KT, P).transpose(0, 3, 2, 1)
        .astype(bf))

    # split-halves RoPE permutation of q/k rows, per head
    perm = np.concatenate([np.arange(0, HD, 2), np.arange(1, HD, 2)])
    full_perm = (np.arange(H)[:, None] * HD + perm[None, :]).reshape(-1)
    wq_p = np.asarray(wq)[full_perm]
    wk_p = np.asarray(wk)[full_perm]

    ccT = np.empty((P, S), np.float32)
    ssT = np.empty((P, S), np.float32)
    ct = np.asarray(freqs_cos).T          # [64, S]
    st = np.asarray(freqs_sin).T
    ccT[0:64], ccT[64:128] = ct, ct
    ssT[0:64], ssT[64:128] = -st, st      # new = q*[c;c] + swap(q)*[-s;s]

    m2 = np.asarray(mask)[0, 0]           # [S, S], mask[i, j]
    mband = np.empty((P, 4 * TCH), np.float32)
    for r in range(4):
        # band tile [jl, il] = mask[il, r*128 + jl]
        mband[:, r * TCH:(r + 1) * TCH] = m2[0:TCH, r * P:(r + 1) * P].T

    in_maps = []
    for c in range(NC):
        osl = slice(c * OC, (c + 1) * OC)
        in_maps.append({
            "xT": xT,
            "wqT": _tile_w(wq_p[osl]),
            "wkT": _tile_w(wk_p[osl]),
            "wvT": _tile_w(np.asarray(wv)[osl]),
            "woT": _tile_w(np.asarray(wo)[osl]),
            "ccT": ccT,
            "ssT": ssT,
            "mband": mband,
        })
    return in_maps


def kernel(x, wq, wk, wv, wo, freqs_cos, freqs_sin, mask, _results_out=None):
    nc = _get_nc()
    in_maps = _prep_inputs(x, wq, wk, wv, wo, freqs_cos, freqs_sin, mask)
    res = run_bass_kernel_spmd(nc, in_maps, core_ids=list(range(NC)))
    if _results_out is not None:
        _results_out.append(res)
    yT = np.concatenate([res.results[c]["out"] for c in range(NC)], axis=0)
    return np.ascontiguousarray(yT.T).reshape(B, S, D).astype(np.float32)


# revision 37
# speedup vs baseline: 1.0140x; 1.0140x over previous
"""Distributed Trainium2 Bass kernel for a full attention layer (prefill).

Reference computation (B=4, S=1024, D=4096, H=32, HD=128, fp32 I/O):
    xq = rope(x @ wq.T), xk = rope(x @ wk.T), xv = x @ wv.T
    out = softmax(causal(xq xk^T / sqrt(HD))) @ xv
    y   = out @ wo.T

Sharding: 8-way tensor parallel over heads (4 heads / core).  Each core
computes Q/K/V projections for its heads, attention, then the output
projection for its 512-row slice of y^T after an AllGather of the
per-head attention outputs.  Matmul operands are cast to bf16 on the
host (fp32 PSUM accumulation on chip).

Layout notes (everything "transposed", i.e. feature-major):
  - x is fed as xT [D, B*S] so projections produce qT/kT [o, tok]
    directly and the attention chain needs no on-chip transposes:
      scoresT[j,i] = kT_tile^T @ qT_chunk        (j keys, i queries)
      softmax over j (partition axis) via exp + ones-matmul column sums
      attn_T[hd,i] = v_tile^T(nat) @ probsT      (v kept token-major)
  - RoPE pairs are split (re | im halves) per head by permuting
    wq/wk rows on the host.  The cross-partition half-swap is done with
    two SBUF->SBUF DMAs (walrus forbids DVE ops whose operands start at
    different partitions), then the rotation is 3 full-width DVE ops:
      new = q * [c;c] + swap(q) * [-s;s]
  - Causal mask: block-skip fully-masked (j,i) tiles; additive band
    mask (4 distinct 128x512 patterns) on diagonal-crossing tiles.
  - Softmax skips max-subtraction: scores ~ N(0,1), |score| < ~7,
    exp is safe in fp32.  1/sum via reciprocal_approx_fast (~18 bits)
    + gpsimd partition_broadcast.

Schedule: P(ch0..7) -> A(b0) A(b1) W(b0) A(b2) W(b1) A(b3) W(b2) W(b3)
with AllGather(b) issued at the end of A(b); W(b) consumes agout(b).
PV work is software-pipelined one head behind scores/exp so the PE
never waits on the Scalar engine's exp.
"""

import math
import os
import sys

import numpy as np

for _p in ("/opt/trn_rl_repo", "/root/.axon_site/_ro/trn_rl_repo"):
    if os.path.isdir(_p) and _p not in sys.path:
        sys.path.insert(0, _p)

import ml_dtypes  # noqa: E402
import concourse.bass as bass  # noqa: E402
import concourse.mybir as mybir  # noqa: E402
import concourse.tile as tile  # noqa: E402
from concourse import bacc  # noqa: E402
from concourse.bass_utils import run_bass_kernel_spmd  # noqa: E402

B, S, D, H = 4, 1024, 4096, 32
HD = D // H            # 128
NC = 8                 # cores
HPC = H // NC          # 4 heads per core
OC = HPC * HD          # 512 output dims per core
NT = B * S             # 4096 tokens
P = 128
KT = D // P            # 32 contraction tiles
KP = 2                 # k-parts per chunk DMA (for startup pipelining)
KTP = KT // KP         # 16 k-tiles per part
TCH = 512              # token chunk (columns per projection matmul)
NCH = NT // TCH        # 8 chunks
SCALE = 1.0 / math.sqrt(HD)

BF16 = mybir.dt.bfloat16
F32 = mybir.dt.float32


def build():
    nc = bacc.Bacc("TRN2", target_bir_lowering=False, debug=False,
                   num_devices=NC)

    # ---- I/O ----
    # x and weights arrive pre-tiled to the exact SBUF image so their
    # DMAs are fully contiguous (16KB/partition descriptors)
    xT_d = nc.dram_tensor("xT", [NCH, P, KT, TCH], BF16,
                          kind="ExternalInput")
    wqT_d = nc.dram_tensor("wqT", [P, KT, OC], BF16, kind="ExternalInput")
    wkT_d = nc.dram_tensor("wkT", [P, KT, OC], BF16, kind="ExternalInput")
    wvT_d = nc.dram_tensor("wvT", [P, KT, OC], BF16, kind="ExternalInput")
    woT_d = nc.dram_tensor("woT", [P, KT, OC], BF16, kind="ExternalInput")
    ccT_d = nc.dram_tensor("ccT", [P, S], F32, kind="ExternalInput")
    ssT_d = nc.dram_tensor("ssT", [P, S], F32, kind="ExternalInput")
    mb_d = nc.dram_tensor("mband", [P, 4 * TCH], F32, kind="ExternalInput")
    out_d = nc.dram_tensor("out", [OC, NT], F32, kind="ExternalOutput")

    # ---- internal DRAM ----
    qT_d = nc.dram_tensor("qT_spill", [OC, NT], BF16)
    kT_d = nc.dram_tensor("kT_spill", [OC, NT], BF16)
    v_d = nc.dram_tensor("v_spill", [NT, OC], BF16)
    agin = [nc.dram_tensor(f"agin{b}", [OC, S], BF16) for b in range(B)]
    agout = [nc.dram_tensor(f"agout{b}", [D, S], BF16, addr_space="Shared")
             for b in range(B)]

    def part(dram_ap, csl, kp):
        """k-part kp of a [D, n] dram tensor column slice as [P, KTP, n]."""
        ksl = slice(kp * KTP * P, (kp + 1) * KTP * P)
        return dram_ap[ksl, csl].rearrange("(k p) n -> p k n", p=P)

    def wpart(dram_ap, kp):
        """k-part kp of a pre-tiled [P, KT, n] weight tensor."""
        return dram_ap[:, kp * KTP:(kp + 1) * KTP, :]

    with tile.TileContext(nc) as tc, \
         tc.tile_pool(name="const", bufs=1) as cpool:
        # constants on the gpsimd DMA queue (off the critical path);
        # const pool stays live across all phases
        ccT = cpool.tile([P, S], F32, tag="cc")
        ssT = cpool.tile([P, S], F32, tag="ss")
        mband = cpool.tile([P, 4 * TCH], F32, tag="mb")
        ones_col = cpool.tile([P, 1], BF16, tag="oc")
        nc.gpsimd.dma_start(ccT[:], ccT_d.ap())
        nc.gpsimd.dma_start(ssT[:], ssT_d.ap())
        nc.gpsimd.dma_start(mband[:], mb_d.ap())
        nc.vector.memset(ones_col[:], 1.0)

        with tc.tile_pool(name="pw", bufs=1) as pw, \
             tc.tile_pool(name="px", bufs=4) as px, \
             tc.tile_pool(name="pr", bufs=3) as pr, \
             tc.tile_pool(name="po", bufs=4) as po, \
             tc.tile_pool(name="pps", bufs=4, space="PSUM") as pps:

            # weights as 2 k-parts each; only wq part0 ahead of the first
            # x chunk (x streams on the Scalar engine's separate HWDGE ring,
            # so weight and x loads run concurrently)
            w_sb = {}
            for kp in range(KP):
                for wname, wd in (("q", wqT_d), ("k", wkT_d), ("v", wvT_d)):
                    t = pw.tile([P, KTP, OC], BF16, tag=f"w{wname}{kp}")
                    w_sb[(wname, kp)] = (t, wd)
            order = [("q", 0), ("q", 1), ("k", 0), ("k", 1), ("v", 0),
                     ("v", 1)]
            for wname, kp in order:
                t, wd = w_sb[(wname, kp)]
                nc.sync.dma_start(t[:], wpart(wd.ap(), kp))
                w_sb[(wname, kp)] = t

            # ---------- phase P: projections + RoPE ----------
            def proj_chunk(ch):
                tsl = slice(ch * TCH, (ch + 1) * TCH)
                psl = slice((ch % 2) * TCH, (ch % 2) * TCH + TCH)
                xc = [px.tile([P, KTP, TCH], BF16, tag="x", name=f"xc{kp}")
                      for kp in range(KP)]
                for kp in range(KP):
                    nc.scalar.dma_start(
                        xc[kp][:],
                        xT_d.ap()[ch, :, kp * KTP:(kp + 1) * KTP, :])

                # q/k projections (transposed out) + RoPE
                for wname, spill in (("q", qT_d), ("k", kT_d)):
                    for ot in range(HPC):
                        osl = slice(ot * P, (ot + 1) * P)
                        ps = pps.tile([P, TCH], F32, tag="ps")
                        for k in range(KT):
                            kp, ki = divmod(k, KTP)
                            nc.tensor.matmul(
                                ps[:], lhsT=w_sb[(wname, kp)][:, ki, osl],
                                rhs=xc[kp][:, ki, :],
                                start=(k == 0), stop=(k == KT - 1))
                        qb = pr.tile([P, TCH], BF16, tag="qb")
                        nc.scalar.copy(qb[:], ps[:])
                        sw = pr.tile([P, TCH], BF16, tag="sw")
                        nc.sync.dma_start(sw[0:64, :], qb[64:128, :])
                        nc.sync.dma_start(sw[64:128, :], qb[0:64, :])
                        qc = pr.tile([P, TCH], F32, tag="qc")
                        qs = pr.tile([P, TCH], F32, tag="qs")
                        nc.vector.tensor_tensor(
                            out=qc[:], in0=ps[:], in1=ccT[:, psl],
                            op=mybir.AluOpType.mult)
                        nc.vector.tensor_tensor(
                            out=qs[:], in0=sw[:], in1=ssT[:, psl],
                            op=mybir.AluOpType.mult)
                        ro = pr.tile([P, TCH], BF16, tag="ro")
                        nc.vector.tensor_tensor(
                            out=ro[:], in0=qc[:], in1=qs[:],
                            op=mybir.AluOpType.add)
                        nc.sync.dma_start(spill.ap()[osl, tsl], ro[:])

                # v projection (token-major out)
                for jt in range(TCH // P):
                    jsl = slice(jt * P, (jt + 1) * P)
                    ps = pps.tile([P, OC], F32, tag="ps")
                    for k in range(KT):
                        kp, ki = divmod(k, KTP)
                        nc.tensor.matmul(
                            ps[:], lhsT=xc[kp][:, ki, jsl],
                            rhs=w_sb[("v", kp)][:, ki, :],
                            start=(k == 0), stop=(k == KT - 1))
                    vo = po.tile([P, OC], BF16, tag="vo")
                    nc.vector.tensor_copy(vo[:], ps[:])
                    nc.sync.dma_start(
                        v_d.ap()[ch * TCH + jt * P:
                                 ch * TCH + (jt + 1) * P, :], vo[:])

            for ch in range(NCH):
                proj_chunk(ch)

        # ---------- phases A (attention) + W (output proj), interleaved ----
        # scores (phase A) and wo (phase W) psum tiles share one pool+tag:
        # they are never live at the same time, so both get 4 banks
        with tc.tile_pool(name="aq", bufs=4) as aq, \
             tc.tile_pool(name="av", bufs=6) as av, \
             tc.tile_pool(name="app", bufs=3) as app, \
             tc.tile_pool(name="ao", bufs=3) as ao, \
             tc.tile_pool(name="asm", bufs=4) as asm, \
             tc.tile_pool(name="ww", bufs=1) as ww, \
             tc.tile_pool(name="wg", bufs=4) as wg, \
             tc.tile_pool(name="wy", bufs=4) as wy, \
             tc.tile_pool(name="aps", bufs=4, space="PSUM") as aps, \
             tc.tile_pool(name="apv", bufs=2, space="PSUM") as apv, \
             tc.tile_pool(name="asu", bufs=2, space="PSUM") as asu:
            wps = aps

            # wo loads on the scalar ring: it is idle at attention start,
            # and the sync ring must stay clear for q/k/v loads
            wo_sb = {}
            for kp in range(KP):
                t = ww.tile([P, KTP, OC], BF16, tag=f"wo{kp}")
                nc.scalar.dma_start(t[:], wpart(woT_d.ap(), kp))
                wo_sb[kp] = t

            def jmax(ic):       # causal: j tiles 0..jmax-1 for i-chunk ic
                return 4 * (ic + 1)

            def mslot(ic, jt):  # probsT slot index
                return jt if ic == 0 else 4 + jt

            def do_scores(b, h):
                qTh = aq.tile([P, S], BF16, tag="q")
                kTh = aq.tile([P, S], BF16, tag="k")
                vh = av.tile([P, S // P, HD], BF16, tag="v")
                hsl = slice(h * P, (h + 1) * P)
                bsl = slice(b * S, (b + 1) * S)
                nc.sync.dma_start(qTh[:], qT_d.ap()[hsl, bsl])
                nc.sync.dma_start(kTh[:], kT_d.ap()[hsl, bsl])
                nc.sync.dma_start(
                    vh[:],
                    v_d.ap()[bsl, hsl].rearrange("(j p) n -> p j n", p=P))
                pp = app.tile([P, 12, TCH], BF16, tag="pp")
                for ic in range(2):
                    for jt in range(jmax(ic)):
                        m = mslot(ic, jt)
                        r = jt - 4 * ic
                        if r < 0:
                            # fully below the diagonal: no mask
                            sps = aps.tile([P, TCH], F32, tag="s")
                            nc.tensor.matmul(
                                sps[:], lhsT=kTh[:, jt * P:(jt + 1) * P],
                                rhs=qTh[:, ic * TCH:(ic + 1) * TCH],
                                start=True, stop=True)
                            nc.scalar.activation(
                                pp[:, m, :], sps[:],
                                mybir.ActivationFunctionType.Exp,
                                scale=SCALE)
                        else:
                            # diagonal band: cols [0,128r) fully masked ->
                            # memset; triangular mask only on the 128-col
                            # diagonal slice; skip matmul on masked cols
                            w = TCH - r * P   # live columns
                            sps = aps.tile([P, TCH], F32, tag="s")
                            nc.tensor.matmul(
                                sps[:, :w], lhsT=kTh[:, jt * P:(jt + 1) * P],
                                rhs=qTh[:, ic * TCH + r * P:(ic + 1) * TCH],
                                start=True, stop=True)
                            nc.vector.tensor_tensor(
                                out=sps[:, 0:P], in0=sps[:, 0:P],
                                in1=mband[:, 0:P],
                                op=mybir.AluOpType.add)
                            nc.scalar.activation(
                                pp[:, m, r * P:], sps[:, :w],
                                mybir.ActivationFunctionType.Exp,
                                scale=SCALE)
                return pp, vh

            def do_pv(b, h, pp, vh):
                at = ao.tile([P, S], BF16, tag="at")
                for ic in range(2):
                    pv = apv.tile([P, TCH], F32, tag="pv")
                    su = asu.tile([1, TCH], F32, tag="su")
                    jm = jmax(ic)
                    # masked-out columns of diagonal tiles were never
                    # written (and contribute zero): accumulate only the
                    # live column range of each probsT tile
                    def live(jt):
                        r = jt - 4 * ic
                        return 0 if r <= 0 else r * P
                    for jt in range(jm):
                        o = live(jt)
                        nc.tensor.matmul(
                            pv[:, o:], lhsT=vh[:, jt, :],
                            rhs=pp[:, mslot(ic, jt), o:],
                            start=(jt == 0), stop=(jt == jm - 1))
                    for jt in range(jm):
                        o = live(jt)
                        nc.tensor.matmul(
                            su[:, o:], lhsT=ones_col[:],
                            rhs=pp[:, mslot(ic, jt), o:],
                            start=(jt == 0), stop=(jt == jm - 1))
                    rec = asm.tile([1, TCH], F32, tag="rec")
                    nc.vector.reciprocal_approx_fast(rec[:], su[:])
                    bcast = asm.tile([P, TCH], F32, tag="bcast")
                    nc.gpsimd.partition_broadcast(bcast[:], rec[:])
                    nc.vector.tensor_tensor(
                        out=at[:, ic * TCH:(ic + 1) * TCH], in0=pv[:],
                        in1=bcast[:], op=mybir.AluOpType.mult)
                nc.sync.dma_start(agin[b].ap()[h * P:(h + 1) * P, :], at[:])
                if h == HPC - 1:
                    nc.gpsimd.collective_compute(
                        "AllGather", mybir.AluOpType.bypass,
                        ins=[agin[b].ap().opt()],
                        outs=[agout[b].ap().opt()],
                        replica_groups=[list(range(NC))])

            def do_attn(b):
                prev = None
                for h in range(HPC):
                    pp, vh = do_scores(b, h)
                    if prev is not None:
                        do_pv(*prev)
                    prev = (b, h, pp, vh)
                do_pv(*prev)

            def do_wo(b):
                for tc2 in range(2):
                    ch = b * 2 + tc2
                    csl = slice(tc2 * TCH, (tc2 + 1) * TCH)
                    agc = [wg.tile([P, KTP, TCH], BF16, tag="ag",
                                   name=f"agc{kp}") for kp in range(KP)]
                    for kp in range(KP):
                        nc.scalar.dma_start(agc[kp][:],
                                            part(agout[b].ap(), csl, kp))
                    for ot in range(HPC):
                        osl = slice(ot * P, (ot + 1) * P)
                        ps = wps.tile([P, TCH], F32, tag="s")
                        for k in range(KT):
                            kp, ki = divmod(k, KTP)
                            nc.tensor.matmul(
                                ps[:], lhsT=wo_sb[kp][:, ki, osl],
                                rhs=agc[kp][:, ki, :],
                                start=(k == 0), stop=(k == KT - 1))
                        yt = wy.tile([P, TCH], F32, tag="y")
                        nc.vector.tensor_copy(yt[:], ps[:])
                        nc.sync.dma_start(
                            out_d.ap()[osl, ch * TCH:(ch + 1) * TCH], yt[:])

            for b in range(B):
                do_attn(b)
            for b in range(B):
                do_wo(b)

    nc.compile()
    return nc


_BUILT = {}


def _get_nc():
    if "nc" not in _BUILT:
        _BUILT["nc"] = build()
    return _BUILT["nc"]


def _tile_w(w_slice):
    """[OC, D] weight slice -> pre-tiled lhsT image [P, KT, OC] bf16."""
    return np.ascontiguousarray(
        w_slice.T.reshape(KT, P, OC).transpose(1, 0, 2)
        .astype(ml_dtypes.bfloat16))


def _prep_inputs(x, wq, wk, wv, wo, freqs_cos, freqs_sin, mask):
    bf = ml_dtypes.bfloat16
    # x -> [NCH, P, KT, TCH] with xtc[ch, p, k, n] = x[512ch+n, 128k+p]
    xT = np.ascontiguousarray(
        np.asarray(x).reshape(NCH, TCH, KT, P).transpose(0, 3, 2, 1)
        .astype(bf))

    # split-halves RoPE permutation of q/k rows, per head
    perm = np.concatenate([np.arange(0, HD, 2), np.arange(1, HD, 2)])
    full_perm = (np.arange(H)[:, None] * HD + perm[None, :]).reshape(-1)
    wq_p = np.asarray(wq)[full_perm]
    wk_p = np.asarray(wk)[full_perm]

    ccT = np.empty((P, S), np.float32)
    ssT = np.empty((P, S), np.float32)
    ct = np.asarray(freqs_cos).T          # [64, S]
    st = np.asarray(freqs_sin).T
    ccT[0:64], ccT[64:128] = ct, ct
    ssT[0:64], ssT[64:128] = -st, st      # new = q*[c;c] + swap(q)*[-s;s]

    m2 = np.asarray(mask)[0, 0]           # [S, S], mask[i, j]
    mband = np.empty((P, 4 * TCH), np.float32)
    for r in range(4):
        # band tile [jl, il] = mask[il, r*128 + jl]
        mband[:, r * TCH:(r + 1) * TCH] = m2[0:TCH, r * P:(r + 1) * P].T

    in_maps = []
    for c in range(NC):
        osl = slice(c * OC, (c + 1) * OC)
        in_maps.append({
            "xT": xT,
            "wqT": _tile_w(wq_p[osl]),
            "wkT": _tile_w(wk_p[osl]),
            "wvT": _tile_w(np.asarray(wv)[osl]),
            "woT": _tile_w(np.asarray(wo)[osl]),
            "ccT": ccT,
            "ssT": ssT,
            "mband": mband,
        })
    return in_maps


def kernel(x, wq, wk, wv, wo, freqs_cos, freqs_sin, mask, _results_out=None):
    nc = _get_nc()
    in_maps = _prep_inputs(x, wq, wk, wv, wo, freqs_cos, freqs_sin, mask)
    res = run_bass_kernel_spmd(nc, in_maps, core_ids=list(range(NC)))
    if _results_out is not None:
        _results_out.append(res)
    yT = np.concatenate([res.results[c]["out"] for c in range(NC)], axis=0)
    return np.ascontiguousarray(yT.T).reshape(B, S, D).astype(np.float32)


# revision 43
# speedup vs baseline: 1.0591x; 1.0444x over previous
"""Distributed Trainium2 Bass kernel for a full attention layer (prefill).

Reference computation (B=4, S=1024, D=4096, H=32, HD=128, fp32 I/O):
    xq = rope(x @ wq.T), xk = rope(x @ wk.T), xv = x @ wv.T
    out = softmax(causal(xq xk^T / sqrt(HD))) @ xv
    y   = out @ wo.T

Sharding: 8-way tensor parallel over heads (4 heads / core).  Each core
computes Q/K/V projections for its heads, attention, then the output
projection for its 512-row slice of y^T after an AllGather of the
per-head attention outputs.  Matmul operands are cast to bf16 on the
host (fp32 PSUM accumulation on chip).

Layout notes (everything "transposed", i.e. feature-major):
  - x is fed as xT [D, B*S] so projections produce qT/kT [o, tok]
    directly and the attention chain needs no on-chip transposes:
      scoresT[j,i] = kT_tile^T @ qT_chunk        (j keys, i queries)
      softmax over j (partition axis) via exp + ones-matmul column sums
      attn_T[hd,i] = v_tile^T(nat) @ probsT      (v kept token-major)
  - RoPE pairs are split (re | im halves) per head by permuting
    wq/wk rows on the host.  The cross-partition half-swap is done with
    two SBUF->SBUF DMAs (walrus forbids DVE ops whose operands start at
    different partitions), then the rotation is 3 full-width DVE ops:
      new = q * [c;c] + swap(q) * [-s;s]
  - Causal mask: block-skip fully-masked (j,i) tiles; additive band
    mask (4 distinct 128x512 patterns) on diagonal-crossing tiles.
  - Softmax skips max-subtraction: scores ~ N(0,1), |score| < ~7,
    exp is safe in fp32.  1/sum via reciprocal_approx_fast (~18 bits)
    + gpsimd partition_broadcast.

Schedule: P(ch0..7) -> A(b0) A(b1) W(b0) A(b2) W(b1) A(b3) W(b2) W(b3)
with AllGather(b) issued at the end of A(b); W(b) consumes agout(b).
PV work is software-pipelined one head behind scores/exp so the PE
never waits on the Scalar engine's exp.
"""

import math
import os
import sys

import numpy as np

for _p in ("/opt/trn_rl_repo", "/root/.axon_site/_ro/trn_rl_repo"):
    if os.path.isdir(_p) and _p not in sys.path:
        sys.path.insert(0, _p)

import ml_dtypes  # noqa: E402
import concourse.bass as bass  # noqa: E402
import concourse.mybir as mybir  # noqa: E402
import concourse.tile as tile  # noqa: E402
from concourse import bacc  # noqa: E402
from concourse.bass_utils import run_bass_kernel_spmd  # noqa: E402

B, S, D, H = 4, 1024, 4096, 32
HD = D // H            # 128
NC = 8                 # cores
HPC = H // NC          # 4 heads per core
OC = HPC * HD          # 512 output dims per core
NT = B * S             # 4096 tokens
P = 128
KT = D // P            # 32 contraction tiles
KP = 2                 # k-parts per chunk DMA (for startup pipelining)
KTP = KT // KP         # 16 k-tiles per part
TCH = 512              # token chunk (columns per projection matmul)
NCH = NT // TCH        # 8 chunks
SCALE = 1.0 / math.sqrt(HD)

BF16 = mybir.dt.bfloat16
F32 = mybir.dt.float32


def build():
    nc = bacc.Bacc("TRN2", target_bir_lowering=False, debug=False,
                   num_devices=NC)

    # ---- I/O ----
    # x and weights arrive pre-tiled to the exact SBUF image so their
    # DMAs are fully contiguous (16KB/partition descriptors)
    xT_d = nc.dram_tensor("xT", [NCH, P, KT, TCH], BF16,
                          kind="ExternalInput")
    wqT_d = nc.dram_tensor("wqT", [P, KT, OC], BF16, kind="ExternalInput")
    wkT_d = nc.dram_tensor("wkT", [P, KT, OC], BF16, kind="ExternalInput")
    wvT_d = nc.dram_tensor("wvT", [P, KT, OC], BF16, kind="ExternalInput")
    woT_d = nc.dram_tensor("woT", [P, KT, OC], BF16, kind="ExternalInput")
    ccT_d = nc.dram_tensor("ccT", [P, S], F32, kind="ExternalInput")
    ssT_d = nc.dram_tensor("ssT", [P, S], F32, kind="ExternalInput")
    mb_d = nc.dram_tensor("mband", [P, 4 * TCH], F32, kind="ExternalInput")
    out_d = nc.dram_tensor("out", [OC, NT], F32, kind="ExternalOutput")

    # ---- internal DRAM ----
    qT_d = nc.dram_tensor("qT_spill", [OC, NT], BF16)
    kT_d = nc.dram_tensor("kT_spill", [OC, NT], BF16)
    v_d = nc.dram_tensor("v_spill", [NT, OC], BF16)
    agin = [nc.dram_tensor(f"agin{b}", [OC, S], BF16) for b in range(B)]
    agout = [nc.dram_tensor(f"agout{b}", [D, S], BF16, addr_space="Shared")
             for b in range(B)]

    def part(dram_ap, csl, kp):
        """k-part kp of a [D, n] dram tensor column slice as [P, KTP, n]."""
        ksl = slice(kp * KTP * P, (kp + 1) * KTP * P)
        return dram_ap[ksl, csl].rearrange("(k p) n -> p k n", p=P)

    def wpart(dram_ap, kp):
        """k-part kp of a pre-tiled [P, KT, n] weight tensor."""
        return dram_ap[:, kp * KTP:(kp + 1) * KTP, :]

    with tile.TileContext(nc) as tc, \
         tc.tile_pool(name="const", bufs=1) as cpool:
        # constants on the gpsimd DMA queue (off the critical path);
        # const pool stays live across all phases
        ccT = cpool.tile([P, S], F32, tag="cc")
        ssT = cpool.tile([P, S], F32, tag="ss")
        mband = cpool.tile([P, 4 * TCH], F32, tag="mb")
        ones_col = cpool.tile([P, 1], BF16, tag="oc")
        nc.gpsimd.dma_start(ccT[:], ccT_d.ap())
        nc.gpsimd.dma_start(ssT[:], ssT_d.ap())
        nc.gpsimd.dma_start(mband[:], mb_d.ap())
        nc.vector.memset(ones_col[:], 1.0)

        with tc.tile_pool(name="pw", bufs=1) as pw, \
             tc.tile_pool(name="px", bufs=4) as px, \
             tc.tile_pool(name="pr", bufs=3) as pr, \
             tc.tile_pool(name="po", bufs=4) as po, \
             tc.tile_pool(name="pps", bufs=4, space="PSUM") as pps:

            # weights as 2 k-parts each; only wq part0 ahead of the first
            # x chunk (x streams on the Scalar engine's separate HWDGE ring,
            # so weight and x loads run concurrently)
            w_sb = {}
            for kp in range(KP):
                for wname, wd in (("q", wqT_d), ("k", wkT_d), ("v", wvT_d)):
                    t = pw.tile([P, KTP, OC], BF16, tag=f"w{wname}{kp}")
                    w_sb[(wname, kp)] = (t, wd)
            order = [("q", 0), ("q", 1), ("k", 0), ("k", 1), ("v", 0),
                     ("v", 1)]
            for wname, kp in order:
                t, wd = w_sb[(wname, kp)]
                nc.sync.dma_start(t[:], wpart(wd.ap(), kp))
                w_sb[(wname, kp)] = t

            # ---------- phase P: projections + RoPE ----------
            def proj_chunk(ch):
                tsl = slice(ch * TCH, (ch + 1) * TCH)
                psl = slice((ch % 2) * TCH, (ch % 2) * TCH + TCH)
                xc = [px.tile([P, KTP, TCH], BF16, tag="x", name=f"xc{kp}")
                      for kp in range(KP)]
                for kp in range(KP):
                    nc.scalar.dma_start(
                        xc[kp][:],
                        xT_d.ap()[ch, :, kp * KTP:(kp + 1) * KTP, :])

                # q/k projections (transposed out) + RoPE
                for wname, spill in (("q", qT_d), ("k", kT_d)):
                    for ot in range(HPC):
                        osl = slice(ot * P, (ot + 1) * P)
                        ps = pps.tile([P, TCH], F32, tag="ps")
                        for k in range(KT):
                            kp, ki = divmod(k, KTP)
                            nc.tensor.matmul(
                                ps[:], lhsT=w_sb[(wname, kp)][:, ki, osl],
                                rhs=xc[kp][:, ki, :],
                                start=(k == 0), stop=(k == KT - 1))
                        qb = pr.tile([P, TCH], BF16, tag="qb")
                        nc.vector.tensor_copy(qb[:], ps[:])
                        sw = pr.tile([P, TCH], BF16, tag="sw")
                        nc.sync.dma_start(sw[0:64, :], qb[64:128, :])
                        nc.sync.dma_start(sw[64:128, :], qb[0:64, :])
                        qc = pr.tile([P, TCH], F32, tag="qc")
                        qs = pr.tile([P, TCH], F32, tag="qs")
                        nc.vector.tensor_tensor(
                            out=qc[:], in0=ps[:], in1=ccT[:, psl],
                            op=mybir.AluOpType.mult)
                        nc.vector.tensor_tensor(
                            out=qs[:], in0=sw[:], in1=ssT[:, psl],
                            op=mybir.AluOpType.mult)
                        ro = pr.tile([P, TCH], BF16, tag="ro")
                        nc.vector.tensor_tensor(
                            out=ro[:], in0=qc[:], in1=qs[:],
                            op=mybir.AluOpType.add)
                        nc.sync.dma_start(spill.ap()[osl, tsl], ro[:])

                # v projection (token-major out)
                for jt in range(TCH // P):
                    jsl = slice(jt * P, (jt + 1) * P)
                    ps = pps.tile([P, OC], F32, tag="ps")
                    for k in range(KT):
                        kp, ki = divmod(k, KTP)
                        nc.tensor.matmul(
                            ps[:], lhsT=xc[kp][:, ki, jsl],
                            rhs=w_sb[("v", kp)][:, ki, :],
                            start=(k == 0), stop=(k == KT - 1))
                    vo = po.tile([P, OC], BF16, tag="vo")
                    nc.vector.tensor_copy(vo[:], ps[:])
                    nc.sync.dma_start(
                        v_d.ap()[ch * TCH + jt * P:
                                 ch * TCH + (jt + 1) * P, :], vo[:])

            for ch in range(NCH):
                proj_chunk(ch)

        # ---------- phases A (attention) + W (output proj), interleaved ----
        # scores (phase A) and wo (phase W) psum tiles share one pool+tag:
        # they are never live at the same time, so both get 4 banks
        with tc.tile_pool(name="aq", bufs=4) as aq, \
             tc.tile_pool(name="av", bufs=6) as av, \
             tc.tile_pool(name="app", bufs=3) as app, \
             tc.tile_pool(name="ao", bufs=3) as ao, \
             tc.tile_pool(name="asm", bufs=4) as asm, \
             tc.tile_pool(name="ww", bufs=1) as ww, \
             tc.tile_pool(name="wg", bufs=4) as wg, \
             tc.tile_pool(name="wy", bufs=4) as wy, \
             tc.tile_pool(name="aps", bufs=4, space="PSUM") as aps, \
             tc.tile_pool(name="apv", bufs=2, space="PSUM") as apv, \
             tc.tile_pool(name="asu", bufs=2, space="PSUM") as asu:
            wps = aps

            # wo loads on the scalar ring: it is idle at attention start,
            # and the sync ring must stay clear for q/k/v loads
            wo_sb = {}
            for kp in range(KP):
                t = ww.tile([P, KTP, OC], BF16, tag=f"wo{kp}")
                nc.scalar.dma_start(t[:], wpart(woT_d.ap(), kp))
                wo_sb[kp] = t

            state = {"last_exp": None}

            def jmax(ic):       # causal: j tiles 0..jmax-1 for i-chunk ic
                return 4 * (ic + 1)

            def mslot(ic, jt):  # probsT slot index
                return jt if ic == 0 else 4 + jt

            def do_scores(b, h):
                qTh = aq.tile([P, S], BF16, tag="q")
                kTh = aq.tile([P, S], BF16, tag="k")
                vh = av.tile([P, S // P, HD], BF16, tag="v")
                hsl = slice(h * P, (h + 1) * P)
                bsl = slice(b * S, (b + 1) * S)
                nc.sync.dma_start(qTh[:], qT_d.ap()[hsl, bsl])
                nc.sync.dma_start(kTh[:], kT_d.ap()[hsl, bsl])
                nc.sync.dma_start(
                    vh[:],
                    v_d.ap()[bsl, hsl].rearrange("(j p) n -> p j n", p=P))
                pp = app.tile([P, 12, TCH], BF16, tag="pp")
                for ic in range(2):
                    for jt in range(jmax(ic)):
                        m = mslot(ic, jt)
                        r = jt - 4 * ic
                        if r < 0:
                            # fully below the diagonal: no mask
                            sps = aps.tile([P, TCH], F32, tag="s")
                            nc.tensor.matmul(
                                sps[:], lhsT=kTh[:, jt * P:(jt + 1) * P],
                                rhs=qTh[:, ic * TCH:(ic + 1) * TCH],
                                start=True, stop=True)
                            state["last_exp"] = nc.scalar.activation(
                                pp[:, m, :], sps[:],
                                mybir.ActivationFunctionType.Exp,
                                scale=SCALE)
                        else:
                            # diagonal band: cols [0,128r) fully masked ->
                            # memset; triangular mask only on the 128-col
                            # diagonal slice; skip matmul on masked cols
                            w = TCH - r * P   # live columns
                            sps = aps.tile([P, TCH], F32, tag="s")
                            nc.tensor.matmul(
                                sps[:, :w], lhsT=kTh[:, jt * P:(jt + 1) * P],
                                rhs=qTh[:, ic * TCH + r * P:(ic + 1) * TCH],
                                start=True, stop=True)
                            nc.vector.tensor_tensor(
                                out=sps[:, 0:P], in0=sps[:, 0:P],
                                in1=mband[:, 0:P],
                                op=mybir.AluOpType.add)
                            state["last_exp"] = nc.scalar.activation(
                                pp[:, m, r * P:], sps[:, :w],
                                mybir.ActivationFunctionType.Exp,
                                scale=SCALE)
                return pp, vh

            def do_pv(b, h, pp, vh):
                at = ao.tile([P, S], BF16, tag="at")
                for ic in range(2):
                    pv = apv.tile([P, TCH], F32, tag="pv")
                    su = asu.tile([1, TCH], F32, tag="su")
                    jm = jmax(ic)
                    # masked-out columns of diagonal tiles were never
                    # written (and contribute zero): accumulate only the
                    # live column range of each probsT tile
                    def live(jt):
                        r = jt - 4 * ic
                        return 0 if r <= 0 else r * P
                    for jt in range(jm):
                        o = live(jt)
                        nc.tensor.matmul(
                            pv[:, o:], lhsT=vh[:, jt, :],
                            rhs=pp[:, mslot(ic, jt), o:],
                            start=(jt == 0), stop=(jt == jm - 1))
                    for jt in range(jm):
                        o = live(jt)
                        nc.tensor.matmul(
                            su[:, o:], lhsT=ones_col[:],
                            rhs=pp[:, mslot(ic, jt), o:],
                            start=(jt == 0), stop=(jt == jm - 1))
                    rec = asm.tile([1, TCH], F32, tag="rec")
                    nc.vector.reciprocal_approx_fast(rec[:], su[:])
                    bcast = asm.tile([P, TCH], F32, tag="bcast")
                    nc.gpsimd.partition_broadcast(bcast[:], rec[:])
                    nc.vector.tensor_tensor(
                        out=at[:, ic * TCH:(ic + 1) * TCH], in0=pv[:],
                        in1=bcast[:], op=mybir.AluOpType.mult)
                nc.sync.dma_start(agin[b].ap()[h * P:(h + 1) * P, :], at[:])
                if h == HPC - 1:
                    nc.gpsimd.collective_compute(
                        "AllGather", mybir.AluOpType.bypass,
                        ins=[agin[b].ap().opt()],
                        outs=[agout[b].ap().opt()],
                        replica_groups=[list(range(NC))])

            def do_attn(b):
                prev = None
                for h in range(HPC):
                    pp, vh = do_scores(b, h)
                    if prev is not None:
                        do_pv(*prev)
                    prev = (b, h, pp, vh)
                do_pv(*prev)

            def do_wo(b):
                for tc2 in range(2):
                    ch = b * 2 + tc2
                    csl = slice(tc2 * TCH, (tc2 + 1) * TCH)
                    agc = [wg.tile([P, KTP, TCH], BF16, tag="ag",
                                   name=f"agc{kp}") for kp in range(KP)]
                    for kp in range(KP):
                        dma = nc.scalar.dma_start(agc[kp][:],
                                                  part(agout[b].ap(), csl,
                                                       kp))
                        # an agc dma waits on its AllGather; if the
                        # scheduler hoists it into the attention-phase ACT
                        # stream, that wait blocks all later exps (the
                        # engine stalls on the dma's wait condition).  Pin
                        # it after the last exp instruction.
                        tile.add_dep_helper(dma.ins, state["last_exp"].ins,
                                            False,
                                            "agc load after attention exps")
                    for ot in range(HPC):
                        osl = slice(ot * P, (ot + 1) * P)
                        ps = wps.tile([P, TCH], F32, tag="s")
                        for k in range(KT):
                            kp, ki = divmod(k, KTP)
                            nc.tensor.matmul(
                                ps[:], lhsT=wo_sb[kp][:, ki, osl],
                                rhs=agc[kp][:, ki, :],
                                start=(k == 0), stop=(k == KT - 1))
                        yt = wy.tile([P, TCH], F32, tag="y")
                        nc.vector.tensor_copy(yt[:], ps[:])
                        nc.sync.dma_start(
                            out_d.ap()[osl, ch * TCH:(ch + 1) * TCH], yt[:])

            for b in range(B):
                do_attn(b)
            for b in range(B):
                do_wo(b)

    nc.compile()
    return nc


_BUILT = {}


def _get_nc():
    if "nc" not in _BUILT:
        _BUILT["nc"] = build()
    return _BUILT["nc"]


def _tile_w(w_slice):
    """[OC, D] weight slice -> pre-tiled lhsT image [P, KT, OC] bf16."""
    return np.ascontiguousarray(
        w_slice.T.reshape(KT, P, OC).transpose(1, 0, 2)
        .astype(ml_dtypes.bfloat16))


def _prep_inputs(x, wq, wk, wv, wo, freqs_cos, freqs_sin, mask):
    bf = ml_dtypes.bfloat16
    # x -> [NCH, P, KT, TCH] with xtc[ch, p, k, n] = x[512ch+n, 128k+p]
    xT = np.ascontiguousarray(
        np.asarray(x).reshape(NCH, TCH, KT, P).transpose(0, 3, 2, 1)
        .astype(bf))

    # split-halves RoPE permutation of q/k rows, per head
    perm = np.concatenate([np.arange(0, HD, 2), np.arange(1, HD, 2)])
    full_perm = (np.arange(H)[:, None] * HD + perm[None, :]).reshape(-1)
    wq_p = np.asarray(wq)[full_perm]
    wk_p = np.asarray(wk)[full_perm]

    ccT = np.empty((P, S), np.float32)
    ssT = np.empty((P, S), np.float32)
    ct = np.asarray(freqs_cos).T          # [64, S]
    st = np.asarray(freqs_sin).T
    ccT[0:64], ccT[64:128] = ct, ct
    ssT[0:64], ssT[64:128] = -st, st      # new = q*[c;c] + swap(q)*[-s;s]

    m2 = np.asarray(mask)[0, 0]           # [S, S], mask[i, j]
    mband = np.empty((P, 4 * TCH), np.float32)
    for r in range(4):
        # band tile [jl, il] = mask[il, r*128 + jl]
        mband[:, r * TCH:(r + 1) * TCH] = m2[0:TCH, r * P:(r + 1) * P].T

    in_maps = []
    for c in range(NC):
        osl = slice(c * OC, (c + 1) * OC)
        in_maps.append({
            "xT": xT,
            "wqT": _tile_w(wq_p[osl]),
            "wkT": _tile_w(wk_p[osl]),
            "wvT": _tile_w(np.asarray(wv)[osl]),
            "woT": _tile_w(np.asarray(wo)[osl]),
            "ccT": ccT,
            "ssT": ssT,
            "mband": mband,
        })
    return in_maps


def kernel(x, wq, wk, wv, wo, freqs_cos, freqs_sin, mask, _results_out=None):
    nc = _get_nc()
    in_maps = _prep_inputs(x, wq, wk, wv, wo, freqs_cos, freqs_sin, mask)
    res = run_bass_kernel_spmd(nc, in_maps, core_ids=list(range(NC)))
    if _results_out is not None:
        _results_out.append(res)
    yT = np.concatenate([res.results[c]["out"] for c in range(NC)], axis=0)
    return np.ascontiguousarray(yT.T).reshape(B, S, D).astype(np.float32)
